# revision 1
# baseline (speedup 1.0000x reference)
"""Slot-attention kernel for Trainium2, SPMD over 8 NeuronCores.

Reference computation (per batch element b):
  query[b,n,:] = q[n,b,:] @ qw[n]          (n = 32 query slots)
  keyp [b,m,:] = k[m,b,:] @ kw[m]          (m = 32 key slots)
  value[b,m,:] = k[m,b,:] @ vw[m]
  logits[b,n,m] = query[b,n,:]·keyp[b,m,:] / 16
  attn = softmax_m(logits)
  out[n,b,:] = sum_m attn[b,n,m] * value[b,m,:]

Sharding: data-parallel over batch (4096 -> 512 per core), weights replicated.
Host pre-casts to bf16 and pre-transposes q/k to [slot, dim, batch] so every
DMA is contiguous and the contraction dim (dim) lands on SBUF partitions.

Per-core schedule (two batch halves of 256; phases B/C per 128-batch
sub-half):
  A) per-slot projections on PE with N=256 moving (full half); K/V weights
     loaded once and kept resident, Q weights streamed per half. Full-bank
     [128,512] psum tiles, one contiguous psum->sbuf copy per slot.
     V lands as [b, sh, m, o] quarter slabs; each completed quarter is
     shuffled into per-sub-half V32Q[32r+m, g, o] via r-quad DMAs on the
     idle GpSimd SWDGE (4 strided partitions per DMA, 32 per sub-half).
  B) logits via col-tiled matmuls (batch = 32j + g within the sub-half),
     16 batches per full psum bank, one exp per bank; rowsums + reciprocals
     feed the output scaling.
  C) DVE 32x32 transposes pack attn^T; attn@value as 4-way diagonal
     tile-packed matmuls, two groups per psum bank; psum->sbuf copies fold
     the softmax normalization; bf16 output DMA on the scalar HWDGE ring
     (host casts back to f32).
"""

import numpy as np
import ml_dtypes

import concourse.bass as bass
from concourse import bacc
import concourse.mybir as mybir
import concourse.tile as tile
from concourse.bass_utils import run_bass_kernel_spmd

BF16 = mybir.dt.bfloat16
F32 = mybir.dt.float32

NQ = 32          # query slots
NK = 32          # key slots
D = 256          # input dim (contraction of projections)
A = 256          # attn dim (contraction of logits)
O = 256          # out dim
BS = 4096
N_CORES = 8
BS_CORE = BS // N_CORES   # 512


def build_kernel(bs_core=BS_CORE, n_halves=2):
    """Builds the per-core Bass graph. bs_core must be divisible by 256."""
    nc = bacc.Bacc()

    b_h = bs_core // n_halves          # batch per half (256)
    b_s = b_h // 2                     # batch per sub-half (128)
    n_groups = b_s // 4                # groups per sub-half (32); b = 32j + g

    qT = nc.declare_dram_parameter("qT", [NQ, D, bs_core], BF16, isOutput=False)
    kT = nc.declare_dram_parameter("kT", [NK, D, bs_core], BF16, isOutput=False)
    qwD = nc.declare_dram_parameter("qw", [NQ, D, A], BF16, isOutput=False)
    # K and V weights merged: [slot, d, 2 (k/v), a]
    kvwD = nc.declare_dram_parameter("kvw", [NK, D, 2, A], BF16,
                                     isOutput=False)
    out = nc.declare_dram_parameter("out", [NQ, bs_core, O], BF16,
                                    isOutput=True)

    SG = 2  # slots per input DMA group
    # [slot, d, b] -> partition = d%128, chunk c = d//128
    qT_g = qT.rearrange("(sg s) (c p) b -> sg p (s c) b", p=128, s=SG)
    kT_g = kT.rearrange("(sg s) (c p) b -> sg p (s c) b", p=128, s=SG)
    qw_g = qwD.rearrange("(sg s) (c p) a -> sg p (s c) a", p=128, s=SG)
    kvw_g = kvwD.rearrange("(sg s) (c p) w a -> sg p (s c) (w a)", p=128, s=SG)

    n_sg = NQ // SG

    with tile.TileContext(nc) as tc:
        with (
            tc.tile_pool(name="const", bufs=1) as const_pool,
            tc.tile_pool(name="win", bufs=2) as win,
            tc.tile_pool(name="xin", bufs=3) as xin,
            tc.tile_pool(name="big", bufs=1) as big,
            tc.tile_pool(name="vnp", bufs=2) as vnp,
            tc.tile_pool(name="vqp", bufs=2) as vqp,
            tc.tile_pool(name="outp", bufs=2) as outp,
            tc.tile_pool(name="smp", bufs=4) as smp,
            tc.tile_pool(name="etp", bufs=6) as etp,
            tc.tile_pool(name="proj_ps", bufs=4, space="PSUM") as proj_ps,
            tc.tile_pool(name="lg_ps", bufs=2, space="PSUM") as lg_ps,
            tc.tile_pool(name="av_ps", bufs=2, space="PSUM") as av_ps,
        ):
            # resident K/V weights: [a-part, slot, c, (kw|vw), a]
            KVW = const_pool.tile([128, NK, 2, 2, A], BF16, tag="KVW")

            # ~5us of dummy back-to-back matmuls while the first input DMAs
            # are in flight, so the PE_HAM clock gate reaches K=8/8 before
            # the real work starts
            warm = const_pool.tile([128, 128], BF16, tag="warm")
            nc.vector.memset(warm[:, :], 0.0)
            wps = av_ps.tile([128, 512], F32, tag="av")
            for _ in range(48):
                nc.tensor.matmul(wps[:, 0:128], lhsT=warm, rhs=warm,
                                 start=True, stop=True)

            for half in range(n_halves):
                b0 = half * b_h
                # ---- Phase A: projections ----
                QTs = big.tile([128, NQ, 2, b_h], BF16, tag="QTs")
                KTs = big.tile([128, NK, 2, b_h], BF16, tag="KTs")
                # V32Q[32r+m, g, o] = value[b0 + sh*128 + 32r + g][m, o]
                V32Q = [vqp.tile([128, n_groups, O], BF16, tag="V32Q",
                                 name=f"V32Q_{half}_{shh}")
                        for shh in range(2)]

                VNq = None
                for sg in range(n_sg):
                    # all input loads on the sync ring (scalar ring would
                    # queue them behind ACT copies; sync is otherwise idle)
                    wsg = win.tile([128, SG, 2, A], BF16, tag="wsg")
                    nc.sync.dma_start(out=wsg, in_=qw_g[sg])
                    qts = xin.tile([128, SG, 2, b_h], BF16, tag="qts")
                    nc.sync.dma_start(out=qts,
                                      in_=qT_g[sg, :, :, b0:b0 + b_h])
                    kts = xin.tile([128, SG, 2, b_h], BF16, tag="kts")
                    nc.sync.dma_start(out=kts,
                                      in_=kT_g[sg, :, :, b0:b0 + b_h])
                    if half == 0:
                        s0 = sg * SG
                        nc.sync.dma_start(
                            out=KVW[:, s0:s0 + SG, :, :, :],
                            in_=kvw_g[sg].rearrange(
                                "p (s c) wa -> p s c wa", s=SG),
                        )

                    if sg % 4 == 0:
                        # value quarter slab [b%128, sub-half, mi, o]
                        VNq = vnp.tile([128, 2, 8, O], BF16, tag="VNq")

                    for si in range(SG):
                        s = sg * SG + si
                        # Q projection: psum [a-tile, 256] full bank
                        ps = proj_ps.tile([128, 2, b_h], F32, tag="ps")
                        for t in range(2):
                            for c in range(2):
                                nc.tensor.matmul(
                                    ps[:, t, :],
                                    lhsT=wsg[:, si, c, t * 128:(t + 1) * 128],
                                    rhs=qts[:, si, c, :],
                                    start=(c == 0),
                                    stop=(c == 1),
                                )
                        nc.scalar.mul(QTs[:, s, :, :], ps, 1.0 / 16.0)
                        # K projection
                        ps = proj_ps.tile([128, 2, b_h], F32, tag="ps")
                        for t in range(2):
                            for c in range(2):
                                nc.tensor.matmul(
                                    ps[:, t, :],
                                    lhsT=KVW[:, s, c, 0,
                                             t * 128:(t + 1) * 128],
                                    rhs=kts[:, si, c, :],
                                    start=(c == 0),
                                    stop=(c == 1),
                                )
                        nc.vector.tensor_copy(out=KTs[:, s, :, :], in_=ps)
                    # V projection: stationary = k batch-chunk, moving = vw
                    # -> psum [b_chunk, si, o]; slot-pair per bank
                    for sh in range(2):
                        ps = proj_ps.tile([128, SG, O], F32, tag="ps")
                        for si in range(SG):
                            s = sg * SG + si
                            for c in range(2):
                                nc.tensor.matmul(
                                    ps[:, si, :],
                                    lhsT=kts[:, si, c,
                                             sh * 128:(sh + 1) * 128],
                                    rhs=KVW[:, s, c, 1, :],
                                    start=(c == 0),
                                    stop=(c == 1),
                                )
                        mrow = (sg * SG) % 8
                        if sh == 0:
                            nc.scalar.copy(
                                out=VNq[:, sh, mrow:mrow + SG, :], in_=ps)
                        else:
                            nc.vector.tensor_copy(
                                out=VNq[:, sh, mrow:mrow + SG, :], in_=ps)

                    # shuffle this sg's freshly written value slots into
                    # the per-sub-half V32Q: row 32r + m <- batch pb = 32r+g
                    # of sub-half sh (one SWDGE DMA per (sh, slot):
                    # 4 strided partitions, 128 KB); fine-grained emission
                    # keeps the end-of-phase tail short
                    for sh in range(2):
                        for si in range(SG):
                            m = sg * SG + si
                            nc.gpsimd.dma_start(
                                out=V32Q[sh][m:m + 97:32, :, :],
                                in_=VNq[:, sh, m % 8, :],
                            )

                for sh in range(2):
                    b0s = b0 + sh * b_s
                    # ---- Phase B: logits + exp + rowsum ----
                    rs = big.tile([128, n_groups], F32, tag="rs")
                    E = big.tile([128, n_groups, NK], BF16, tag="E")

                    for gb in range(n_groups // 16):
                        lg = lg_ps.tile([128, 16, NK], F32, tag="lg")
                        for qi in range(16):
                            g = 16 * gb + qi
                            bl = sh * b_s + g  # batch col in QTs/KTs, j adds 32
                            for c in range(2):
                                for j in range(4):
                                    nc.tensor.matmul(
                                        lg[32 * j:32 * (j + 1), qi, :],
                                        lhsT=QTs[:, :, c, bl + 32 * j],
                                        rhs=KTs[:, :, c, bl + 32 * j],
                                        start=(c == 0),
                                        stop=(c == 1),
                                        tile_position=(0, 32 * j),
                                        skip_group_check=True,
                                    )
                        # softmax over m without max-subtraction: logits
                        # carry the 1/16 so |logit| <= ~2 and exp cannot
                        # overflow; normalization folds into the output copy
                        nc.scalar.activation(
                            out=E[:, 16 * gb:16 * gb + 16, :].rearrange(
                                "p a b -> p (a b)"),
                            in_=lg.rearrange("p a b -> p (a b)"),
                            func=mybir.ActivationFunctionType.Exp,
                        )
                        sm = smp.tile([128, 16], F32, tag="sm")
                        nc.vector.reduce_sum(
                            out=sm, in_=E[:, 16 * gb:16 * gb + 16, :],
                            axis=mybir.AxisListType.X,
                        )
                        nc.vector.reciprocal(out=rs[:, 16 * gb:16 * gb + 16],
                                             in_=sm)
                        # normalize E in place (attn = exp * 1/rowsum) so
                        # phase C copies need no per-group scaling
                        nc.vector.tensor_mul(
                            out=E[:, 16 * gb:16 * gb + 16, :],
                            in0=E[:, 16 * gb:16 * gb + 16, :],
                            in1=rs[:, 16 * gb:16 * gb + 16].unsqueeze(
                                2).to_broadcast([128, 16, NK]),
                        )

                    # ---- Phase C: attn @ value ----
                    g_chunk = 8
                    for g0 in range(0, n_groups, g_chunk):
                        OUTo = outp.tile([128, g_chunk, O], BF16, tag="OUTo")
                        # one DVE pass transposes all 8 groups' 32x32
                        # attn blocks: te8[32j+m, 32gi+n] = E[32j+n, g0+gi, m]
                        te8 = etp.tile([128, g_chunk, NK], BF16, tag="te8")
                        nc.vector.transpose(
                            out=te8.rearrange("p a b -> p (a b)"),
                            in_=E[:, g0:g0 + g_chunk, :].rearrange(
                                "p a b -> p (a b)"))
                        for gp in range(g_chunk // 2):
                            g = g0 + 2 * gp
                            av = av_ps.tile([128, 2, O], F32, tag="av")
                            for gg in range(2):
                                for j in range(4):
                                    nc.tensor.matmul(
                                        av[32 * j:32 * (j + 1), gg, :],
                                        lhsT=te8[32 * j:32 * (j + 1),
                                                 2 * gp + gg, :],
                                        rhs=V32Q[sh][32 * j:32 * (j + 1),
                                                     g + gg, :],
                                        start=True, stop=True,
                                        tile_position=(32 * j, 32 * j),
                                        skip_group_check=True,
                                    )
                            # plain full-bank psum -> sbuf copy
                            if gp % 2 == 0:
                                nc.scalar.copy(
                                    out=OUTo[:, 2 * gp:2 * gp + 2, :],
                                    in_=av)
                            else:
                                nc.vector.tensor_copy(
                                    out=OUTo[:, 2 * gp:2 * gp + 2, :],
                                    in_=av)
                        # flush on the scalar HWDGE ring (sync ring carries
                        # the input loads)
                        for j in range(4):
                            nc.scalar.dma_start(
                                out=out[:, b0s + 32 * j + g0:
                                        b0s + 32 * j + g0 + g_chunk, :],
                                in_=OUTo[32 * j:32 * (j + 1), :, :],
                            )
    return nc


def _prep_inputs(q, k, query_weight, key_weight, value_weight, bs_core):
    bf = ml_dtypes.bfloat16
    qw = np.ascontiguousarray(query_weight).astype(bf)
    kvw = np.ascontiguousarray(
        np.stack((key_weight, value_weight), axis=2)).astype(bf)
    in_maps = []
    for i in range(N_CORES):
        sl = slice(i * bs_core, (i + 1) * bs_core)
        qTb = np.ascontiguousarray(q[:, sl, :].transpose(0, 2, 1)).astype(bf)
        kTb = np.ascontiguousarray(k[:, sl, :].transpose(0, 2, 1)).astype(bf)
        in_maps.append({"qT": qTb, "kT": kTb, "qw": qw, "kvw": kvw})
    return in_maps


_NC_CACHE = {}


def _get_nc(bs_core, n_halves=2):
    key = (bs_core, n_halves)
    if key not in _NC_CACHE:
        nc = build_kernel(bs_core, n_halves)
        nc.finalize()
        _NC_CACHE[key] = nc
    return _NC_CACHE[key]


def kernel(q, k, query_weight, key_weight, value_weight, _trace=False):
    nc = _get_nc(BS_CORE)
    in_maps = _prep_inputs(q, k, query_weight, key_weight, value_weight, BS_CORE)
    res = run_bass_kernel_spmd(nc, in_maps, core_ids=list(range(N_CORES)),
                               trace=_trace)
    outs = [res.results[i]["out"] for i in range(N_CORES)]
    full = np.concatenate(outs, axis=1).astype(np.float32)
    if _trace:
        return full, res
    return full



# revision 2
# speedup vs baseline: 1.0642x; 1.0642x over previous
"""Slot-attention kernel for Trainium2, SPMD over 8 NeuronCores.

Reference computation (per batch element b):
  query[b,n,:] = q[n,b,:] @ qw[n]          (n = 32 query slots)
  keyp [b,m,:] = k[m,b,:] @ kw[m]          (m = 32 key slots)
  value[b,m,:] = k[m,b,:] @ vw[m]
  logits[b,n,m] = query[b,n,:]·keyp[b,m,:] / 16
  attn = softmax_m(logits)
  out[n,b,:] = sum_m attn[b,n,m] * value[b,m,:]

Sharding: data-parallel over batch (4096 -> 512 per core), weights replicated.

Host-side prep packs everything into DMA-contiguous layouts (large per-
partition runs) so every DMA moves >=384KB of linear DRAM:
  combo[h, slot, p, kind, c, x]: per (half, slot) one 384KB DMA carrying
    {qT-slice, kT-slice, qw/16} with d%128 on partitions.
  kvw[g4, p, s, c, w, a]: key/value weights, resident in SBUF (64KB/part).
  out: written back as [h, sh, gc, p, gg, o] (contiguous 512KB stores);
    host untangles to [nq, bs, o].

Per-core schedule (two 256-batch halves; phases B/C per 128-batch sub-half):
  A) per-slot projections, full-clock N=256 matmuls; Q/K psum -> SBUF slot
     copies (ACT/DVE); V psum pairs -> VO[b][o,m] via strided-out copy, then
     one DVE 32x32 block-transpose per sub-half gives V32T[32r+m][o,g]
     (value with key-slot on partitions) -- no SWDGE shuffle.
  B) logits via 4x col-tiled 32x32 matmuls (batch = 32j+g), exp on ACT,
     rowsum+recip+normalize on DVE.
  C) attn^T via DVE 32x32 transpose; attn@value as 4-way diagonal-tiled
     matmuls with strided V32T rhs; psum quad copies -> OUTo; 512KB
     SWDGE stores on the otherwise-idle GpSimd queue.
"""

import numpy as np
import ml_dtypes

import concourse.bass as bass
from concourse import bacc
import concourse.mybir as mybir
import concourse.tile as tile
from concourse.bass_utils import run_bass_kernel_spmd

BF16 = mybir.dt.bfloat16
F32 = mybir.dt.float32

NQ = 32          # query slots
NK = 32          # key slots
D = 256          # input dim (contraction of projections)
A = 256          # attn dim (contraction of logits)
O = 256          # out dim
BS = 4096
N_CORES = 8
BS_CORE = BS // N_CORES   # 512
B_H = 256                 # batch per half
B_S = 128                 # batch per sub-half


def build_kernel():
    nc = bacc.Bacc()

    # combo[h, slot, p, kind(q,k,qw), c, 256]
    comboD = nc.declare_dram_parameter("combo", [2, NQ, 128, 3, 2, 256], BF16,
                                       isOutput=False)
    # kvw[g4, p, s4, c, w(k,v), a]
    kvwD = nc.declare_dram_parameter("kvw", [8, 128, 4, 2, 2, A], BF16,
                                     isOutput=False)
    # out[h, sh, gc, p(32j+n), gg, o]
    outD = nc.declare_dram_parameter("out", [2, 2, 4, 128, 8, O], BF16,
                                     isOutput=True)

    with tile.TileContext(nc) as tc:
        with (
            tc.tile_pool(name="const", bufs=1) as const_pool,
            tc.tile_pool(name="xin", bufs=2) as xin,
            tc.tile_pool(name="big", bufs=1) as big,
            tc.tile_pool(name="vop", bufs=2) as vop,
            tc.tile_pool(name="v32p", bufs=2) as v32p,
            tc.tile_pool(name="outp", bufs=1) as outp,
            tc.tile_pool(name="ep", bufs=1) as ep,
            tc.tile_pool(name="tep", bufs=2) as tep,
            tc.tile_pool(name="rsp", bufs=2) as rsp,
            tc.tile_pool(name="smp", bufs=2) as smp,
            tc.tile_pool(name="qk_ps", bufs=2, space="PSUM") as qk_ps,
            tc.tile_pool(name="vp_ps", bufs=2, space="PSUM") as vp_ps,
            tc.tile_pool(name="lg_ps", bufs=2, space="PSUM") as lg_ps,
        ):
            # resident K/V weights: [p, slot, c, (kw|vw), a]
            KVW = const_pool.tile([128, NK, 2, 2, A], BF16, tag="KVW")

            # ~4us of dummy back-to-back matmuls while the first input DMAs
            # are in flight, so the PE_HAM clock gate reaches K=8/8 before
            # the real work starts
            warm = tep.tile([128, 8, 32], BF16, tag="te8", name="warm")
            nc.vector.memset(warm.rearrange("p a b -> p (a b)"), 0.0)
            wsrc = warm.rearrange("p a b -> p (a b)")[:, 0:128]
            wps = lg_ps.tile([128, 16, 32], F32, tag="lg", name="warmps")
            for _ in range(44):
                nc.tensor.matmul(wps[:, 0:4, :], lhsT=wsrc, rhs=wsrc,
                                 start=True, stop=True)

            # K/V weight residency loads (1MB each) on the scalar HWDGE ring
            for gk in range(8):
                nc.scalar.dma_start(
                    out=KVW[:, 4 * gk:4 * gk + 4, :, :, :], in_=kvwD[gk])

            for h in range(2):
                # ---- Phase A: projections ----
                QTs = big.tile([128, NQ, 2, B_H], BF16, tag="QTs")
                KTs = big.tile([128, NK, 2, B_H], BF16, tag="KTs")
                # VO[sh][b%128, o, m] = value[b][m, o]  (pre-transpose)
                VO = [vop.tile([128, O, NK], BF16, tag="VO",
                               name=f"VO_{h}_{sh}") for sh in range(2)]

                vps = None
                for g in range(NQ):
                    cb = xin.tile([128, 3, 2, 256], BF16, tag="cb")
                    nc.sync.dma_start(out=cb, in_=comboD[h, g])

                    # Q projection: psum [a%128, t, b], qw pre-scaled by 1/16
                    qps = qk_ps.tile([128, 2, B_H], F32, tag="qk")
                    for t in range(2):
                        for c in range(2):
                            nc.tensor.matmul(
                                qps[:, t, :],
                                lhsT=cb[:, 2, c, t * 128:(t + 1) * 128],
                                rhs=cb[:, 0, c, :],
                                start=(c == 0), stop=(c == 1))
                    nc.scalar.copy(out=QTs[:, g, :, :], in_=qps)
                    # K projection
                    kps = qk_ps.tile([128, 2, B_H], F32, tag="qk")
                    for t in range(2):
                        for c in range(2):
                            nc.tensor.matmul(
                                kps[:, t, :],
                                lhsT=KVW[:, g, c, 0, t * 128:(t + 1) * 128],
                                rhs=cb[:, 1, c, :],
                                start=(c == 0), stop=(c == 1))
                    nc.vector.tensor_copy(out=KTs[:, g, :, :], in_=kps)
                    # V projection: psum [b%128, s-pair, sh, o]
                    sp = g % 2
                    if sp == 0:
                        vps = vp_ps.tile([128, 2, 2, O], F32, tag="vp")
                    for sh in range(2):
                        for c in range(2):
                            nc.tensor.matmul(
                                vps[:, sp, sh, :],
                                lhsT=cb[:, 1, c, sh * 128:(sh + 1) * 128],
                                rhs=KVW[:, g, c, 1, :],
                                start=(c == 0), stop=(c == 1))
                    if sp == 1:
                        g0 = g - 1
                        # interleaved copies: VO[sh][:, o, g0+s] = v[s, sh, o]
                        nc.scalar.copy(
                            out=VO[0][:, :, g0:g0 + 2].rearrange(
                                "p o s -> p s o"),
                            in_=vps[:, :, 0, :])
                        nc.vector.tensor_copy(
                            out=VO[1][:, :, g0:g0 + 2].rearrange(
                                "p o s -> p s o"),
                            in_=vps[:, :, 1, :])

                for sh in range(2):
                    # V32T[32r+m][o, g] = VO[32r+g][o, m]
                    V32T = v32p.tile([128, O, NK], BF16, tag="v32t",
                                     name=f"V32T_{h}_{sh}")
                    nc.vector.transpose(
                        out=V32T.rearrange("p a b -> p (a b)"),
                        in_=VO[sh].rearrange("p a b -> p (a b)"))

                    # ---- Phase B: logits + exp + rowsum + normalize ----
                    E = ep.tile([128, 32, NK], BF16, tag="E")   # [g, m]
                    rs = rsp.tile([128, 32], F32, tag="rs")
                    for gb in range(2):
                        lg = lg_ps.tile([128, 16, NK], F32, tag="lg")
                        for qi in range(16):
                            bl = sh * B_S + 16 * gb + qi
                            for t in range(2):
                                for j in range(4):
                                    nc.tensor.matmul(
                                        lg[32 * j:32 * (j + 1), qi, :],
                                        lhsT=QTs[:, :, t, bl + 32 * j],
                                        rhs=KTs[:, :, t, bl + 32 * j],
                                        start=(t == 0), stop=(t == 1),
                                        tile_position=(0, 32 * j),
                                        skip_group_check=True)
                        # softmax over m without max-subtraction: logits
                        # carry the 1/16 so |logit| <= ~2 and exp cannot
                        # overflow
                        sl = slice(16 * gb, 16 * gb + 16)
                        nc.scalar.activation(
                            out=E[:, sl, :].rearrange("p a b -> p (a b)"),
                            in_=lg.rearrange("p a b -> p (a b)"),
                            func=mybir.ActivationFunctionType.Exp)
                        sm = smp.tile([128, 16], F32, tag="sm")
                        nc.vector.reduce_sum(out=sm, in_=E[:, sl, :],
                                             axis=mybir.AxisListType.X)
                        nc.vector.reciprocal(out=rs[:, sl], in_=sm)
                        nc.vector.tensor_mul(
                            out=E[:, sl, :], in0=E[:, sl, :],
                            in1=rs[:, sl].unsqueeze(2).to_broadcast(
                                [128, 16, NK]))

                    # ---- Phase C: attn @ value ----
                    for gc in range(4):
                        OUTo = outp.tile([128, 8, O], BF16, tag="OUTo")
                        te8 = tep.tile([128, 8, NK], BF16, tag="te8")
                        nc.vector.transpose(
                            out=te8.rearrange("p a b -> p (a b)"),
                            in_=E[:, 8 * gc:8 * gc + 8, :].rearrange(
                                "p a b -> p (a b)"))
                        for gq in range(2):
                            av = vp_ps.tile([128, 4, O], F32, tag="vp",
                                            name="av")
                            for gg in range(4):
                                gi = 4 * gq + gg
                                for j in range(4):
                                    nc.tensor.matmul(
                                        av[32 * j:32 * (j + 1), gg, :],
                                        lhsT=te8[32 * j:32 * (j + 1), gi, :],
                                        rhs=V32T[32 * j:32 * (j + 1), :,
                                                 8 * gc + gi],
                                        start=True, stop=True,
                                        tile_position=(32 * j, 32 * j),
                                        skip_group_check=True)
                            nc.scalar.copy(
                                out=OUTo[:, 4 * gq:4 * gq + 4, :], in_=av)
                        nc.gpsimd.dma_start(out=outD[h, sh, gc], in_=OUTo)
    return nc


def _prep_inputs(q, k, query_weight, key_weight, value_weight):
    bf = ml_dtypes.bfloat16
    # combo[h, slot, p, kind, c, x]; kind 0=qT, 1=kT, 2=qw/16
    def pack_qk(x, core):
        # x[32, 4096, 256] -> core slice -> [h, slot, p, c, b]
        xc = x[:, core * BS_CORE:(core + 1) * BS_CORE, :]
        r = xc.reshape(NQ, 2, B_H, 2, 128)        # [s, h, b, c, p]
        return r.transpose(1, 0, 4, 3, 2)          # [h, s, p, c, b]

    qwp = (np.asarray(query_weight) / 16.0).reshape(NQ, 2, 128, A)
    qwp = qwp.transpose(0, 2, 1, 3)                # [s, p, c, a]

    kv = np.stack([np.asarray(key_weight), np.asarray(value_weight)], axis=2)
    # kv[s, d, w, a] -> [g4, p, s4, c, w, a]
    kvp = kv.reshape(8, 4, 2, 128, 2, A).transpose(0, 3, 1, 2, 4, 5)
    kvp = np.ascontiguousarray(kvp).astype(bf)

    in_maps = []
    for i in range(N_CORES):
        qp = pack_qk(np.asarray(q), i)
        kp = pack_qk(np.asarray(k), i)
        combo = np.empty((2, NQ, 128, 3, 2, 256), dtype=bf)
        combo[:, :, :, 0] = qp
        combo[:, :, :, 1] = kp
        combo[:, :, :, 2] = qwp[None]
        in_maps.append({"combo": combo, "kvw": kvp})
    return in_maps


def _unpack_out(outs):
    # per-core out [2 h, 2 sh, 4 gc, 128(32j+n), 8 gg, 256] ->
    # [nq, 512, 256] with b = h*256 + sh*128 + 32j + 8*gc + gg
    full = []
    for o in outs:
        od = np.asarray(o).reshape(2, 2, 4, 4, 32, 8, O)
        full.append(od.transpose(4, 0, 1, 3, 2, 5, 6).reshape(NQ, BS_CORE, O))
    return np.concatenate(full, axis=1).astype(np.float32)


_NC_CACHE = {}


def _get_nc():
    if "nc" not in _NC_CACHE:
        nc = build_kernel()
        nc.finalize()
        _NC_CACHE["nc"] = nc
    return _NC_CACHE["nc"]


def kernel(q, k, query_weight, key_weight, value_weight, _trace=False):
    nc = _get_nc()
    in_maps = _prep_inputs(q, k, query_weight, key_weight, value_weight)
    res = run_bass_kernel_spmd(nc, in_maps, core_ids=list(range(N_CORES)),
                               trace=_trace)
    full = _unpack_out([res.results[i]["out"] for i in range(N_CORES)])
    if _trace:
        return full, res
    return full


# revision 6
# speedup vs baseline: 1.1009x; 1.0345x over previous
"""Slot-attention kernel for Trainium2, SPMD over 8 NeuronCores.

Reference computation (per batch element b):
  query[b,n,:] = q[n,b,:] @ qw[n]          (n = 32 query slots)
  keyp [b,m,:] = k[m,b,:] @ kw[m]          (m = 32 key slots)
  value[b,m,:] = k[m,b,:] @ vw[m]
  logits[b,n,m] = query[b,n,:]·keyp[b,m,:] / 16
  attn = softmax_m(logits)
  out[n,b,:] = sum_m attn[b,n,m] * value[b,m,:]

Sharding: data-parallel over batch (4096 -> 512 per core), weights replicated.

Host-side prep packs everything into DMA-contiguous layouts (large per-
partition runs) so every DMA moves >=384KB of linear DRAM:
  combo[h, slot, p, kind, c, x]: per (half, slot) one 384KB DMA carrying
    {qT-slice, kT-slice, qw/16} with d%128 on partitions.
  kvw[g4, p, s, c, w, a]: key/value weights, resident in SBUF (64KB/part).
  out: written back as [h, sh, gc, p, gg, o] (contiguous 512KB stores);
    host untangles to [nq, bs, o].

Per-core schedule (two 256-batch halves; phases B/C per 128-batch sub-half):
  A) per-slot projections, full-clock N=256 matmuls; Q/K psum -> SBUF slot
     copies (ACT/DVE); V psum pairs -> VO[b][o,m] via strided-out copy, then
     one DVE 32x32 block-transpose per sub-half gives V32T[32r+m][o,g]
     (value with key-slot on partitions) -- no SWDGE shuffle.
  B) logits via 4x col-tiled 32x32 matmuls (batch = 32j+g), exp on ACT,
     rowsum+recip+normalize on DVE.
  C) attn^T via DVE 32x32 transpose; attn@value as 4-way diagonal-tiled
     matmuls with strided V32T rhs; psum quad copies -> OUTo; 512KB
     SWDGE stores on the otherwise-idle GpSimd queue.
"""

import numpy as np
import ml_dtypes

import concourse.bass as bass
from concourse import bacc
import concourse.mybir as mybir
import concourse.tile as tile
from concourse.bass_utils import run_bass_kernel_spmd

BF16 = mybir.dt.bfloat16
F32 = mybir.dt.float32

NQ = 32          # query slots
NK = 32          # key slots
D = 256          # input dim (contraction of projections)
A = 256          # attn dim (contraction of logits)
O = 256          # out dim
BS = 4096
N_CORES = 8
BS_CORE = BS // N_CORES   # 512
B_H = 256                 # batch per half
B_S = 128                 # batch per sub-half


def build_kernel():
    nc = bacc.Bacc()

    # combo[h, slot, p, kind(q,k,qw), c, 256]
    comboD = nc.declare_dram_parameter("combo", [2, NQ, 128, 3, 2, 256], BF16,
                                       isOutput=False)
    # kvw[g4, p, s4, c, w(k,v), a]
    kvwD = nc.declare_dram_parameter("kvw", [8, 128, 4, 2, 2, A], BF16,
                                     isOutput=False)
    # out[h, sh, gc, p(32j+n), gg, o]
    outD = nc.declare_dram_parameter("out", [2, 2, 4, 128, 8, O], BF16,
                                     isOutput=True)

    with tile.TileContext(nc) as tc:
        with (
            tc.tile_pool(name="const", bufs=1) as const_pool,
            tc.tile_pool(name="xin", bufs=2) as xin,
            tc.tile_pool(name="big", bufs=1) as big,
            tc.tile_pool(name="vop", bufs=2) as vop,
            tc.tile_pool(name="v32p", bufs=2) as v32p,
            tc.tile_pool(name="outp", bufs=1) as outp,
            tc.tile_pool(name="ep", bufs=1) as ep,
            tc.tile_pool(name="tep", bufs=2) as tep,
            tc.tile_pool(name="rsp", bufs=2) as rsp,
            tc.tile_pool(name="smp", bufs=2) as smp,
            tc.tile_pool(name="qk_ps", bufs=2, space="PSUM") as qk_ps,
            tc.tile_pool(name="vp_ps", bufs=2, space="PSUM") as vp_ps,
            tc.tile_pool(name="lg_ps", bufs=2, space="PSUM") as lg_ps,
        ):
            # resident K/V weights: [p, slot, c, (kw|vw), a]
            KVW = const_pool.tile([128, NK, 2, 2, A], BF16, tag="KVW")

            # ~4us of dummy back-to-back matmuls while the first input DMAs
            # are in flight, so the PE_HAM clock gate reaches K=8/8 before
            # the real work starts
            warm = tep.tile([128, 8, 32], BF16, tag="te8", name="warm")
            nc.vector.memset(warm.rearrange("p a b -> p (a b)"), 0.0)
            wsrc = warm.rearrange("p a b -> p (a b)")[:, 0:128]
            wps = lg_ps.tile([128, 16, 32], F32, tag="lg", name="warmps")
            for _ in range(44):
                nc.tensor.matmul(wps[:, 0:4, :], lhsT=wsrc, rhs=wsrc,
                                 start=True, stop=True)

            # K/V weight residency loads (1MB each) on the scalar HWDGE ring
            for gk in range(8):
                nc.scalar.dma_start(
                    out=KVW[:, 4 * gk:4 * gk + 4, :, :, :], in_=kvwD[gk])

            for h in range(2):
                # ---- Phase A: projections ----
                QTs = big.tile([128, NQ, 2, B_H], BF16, tag="QTs")
                KTs = big.tile([128, NK, 2, B_H], BF16, tag="KTs")
                # VN[sh][b%128, m, o] = value[b][m, o]  (natural layout)
                VN = [vop.tile([128, NK, O], BF16, tag="VN",
                               name=f"VN_{h}_{sh}") for sh in range(2)]

                vps = None
                for g in range(NQ):
                    cb = xin.tile([128, 3, 2, 256], BF16, tag="cb")
                    # alternate HWDGE(sync) / SWDGE(gpsimd) queues for
                    # input-stream parallelism
                    if g % 2 == 0:
                        nc.sync.dma_start(out=cb, in_=comboD[h, g])
                    else:
                        nc.gpsimd.dma_start(out=cb, in_=comboD[h, g])

                    # Q projection: psum [a%128, t, b], qw pre-scaled by 1/16
                    qps = qk_ps.tile([128, 2, B_H], F32, tag="qk")
                    for t in range(2):
                        for c in range(2):
                            nc.tensor.matmul(
                                qps[:, t, :],
                                lhsT=cb[:, 2, c, t * 128:(t + 1) * 128],
                                rhs=cb[:, 0, c, :],
                                start=(c == 0), stop=(c == 1))
                    nc.scalar.copy(out=QTs[:, g, :, :], in_=qps)
                    # K projection
                    kps = qk_ps.tile([128, 2, B_H], F32, tag="qk")
                    for t in range(2):
                        for c in range(2):
                            nc.tensor.matmul(
                                kps[:, t, :],
                                lhsT=KVW[:, g, c, 0, t * 128:(t + 1) * 128],
                                rhs=cb[:, 1, c, :],
                                start=(c == 0), stop=(c == 1))
                    nc.vector.tensor_copy(out=KTs[:, g, :, :], in_=kps)
                    # V projection: psum [b%128, s-pair, sh, o]
                    sp = g % 2
                    if sp == 0:
                        vps = vp_ps.tile([128, 2, 2, O], F32, tag="vp")
                    for sh in range(2):
                        for c in range(2):
                            nc.tensor.matmul(
                                vps[:, sp, sh, :],
                                lhsT=cb[:, 1, c, sh * 128:(sh + 1) * 128],
                                rhs=KVW[:, g, c, 1, :],
                                start=(c == 0), stop=(c == 1))
                    if sp == 1:
                        g0 = g - 1
                        # contiguous pair copies into the natural V layout
                        nc.scalar.copy(out=VN[0][:, g0:g0 + 2, :],
                                       in_=vps[:, :, 0, :])
                        nc.vector.tensor_copy(out=VN[1][:, g0:g0 + 2, :],
                                              in_=vps[:, :, 1, :])

                # V32T[32r+m][o, g] = VN[32r+g][m, o]: DVE 32x32 block
                # transpose with a strided-read AP (o-major over the natural
                # [m, o] layout). Chunked into 4 ops per sub-half and
                # emission-interleaved with phases B/C below so the DVE FIFO
                # never blocks the softmax chain.
                V32Tt = [v32p.tile([128, O, NK], BF16, tag="v32t",
                                   name=f"V32T_{h}_{sh}") for sh in range(2)]

                def v32t_chunk(sh, oc):
                    nc.vector.transpose(
                        out=V32Tt[sh][:, 64 * oc:64 * (oc + 1), :],
                        in_=VN[sh][:, :, 64 * oc:64 * (oc + 1)].rearrange(
                            "p m o -> p o m"))

                for sh in range(2):
                    V32T = V32Tt[sh]
                    # ---- Phase B: logits + exp + rowsum + normalize ----
                    E = ep.tile([128, 32, NK], BF16, tag="E")   # [g, m]
                    rs = rsp.tile([128, 32], F32, tag="rs")
                    for gb in range(2):
                        lg = lg_ps.tile([128, 16, NK], F32, tag="lg")
                        for qi in range(16):
                            bl = sh * B_S + 16 * gb + qi
                            for t in range(2):
                                for j in range(4):
                                    nc.tensor.matmul(
                                        lg[32 * j:32 * (j + 1), qi, :],
                                        lhsT=QTs[:, :, t, bl + 32 * j],
                                        rhs=KTs[:, :, t, bl + 32 * j],
                                        start=(t == 0), stop=(t == 1),
                                        tile_position=(0, 32 * j),
                                        skip_group_check=True)
                        # softmax over m without max-subtraction: logits
                        # carry the 1/16 so |logit| <= ~2 and exp cannot
                        # overflow
                        sl = slice(16 * gb, 16 * gb + 16)
                        nc.scalar.activation(
                            out=E[:, sl, :].rearrange("p a b -> p (a b)"),
                            in_=lg.rearrange("p a b -> p (a b)"),
                            func=mybir.ActivationFunctionType.Exp)
                        sm = smp.tile([128, 16], F32, tag="sm")
                        nc.vector.reduce_sum(out=sm, in_=E[:, sl, :],
                                             axis=mybir.AxisListType.X)
                        nc.vector.reciprocal(out=rs[:, sl], in_=sm)
                        nc.vector.tensor_mul(
                            out=E[:, sl, :], in0=E[:, sl, :],
                            in1=rs[:, sl].unsqueeze(2).to_broadcast(
                                [128, 16, NK]))
                        if sh == 0:
                            # own V32T chunks between the softmax chains
                            v32t_chunk(0, 2 * gb)
                            v32t_chunk(0, 2 * gb + 1)

                    # ---- Phase C: attn @ value ----
                    for gc in range(4):
                        OUTo = outp.tile([128, 8, O], BF16, tag="OUTo")
                        te8 = tep.tile([128, 8, NK], BF16, tag="te8")
                        nc.vector.transpose(
                            out=te8.rearrange("p a b -> p (a b)"),
                            in_=E[:, 8 * gc:8 * gc + 8, :].rearrange(
                                "p a b -> p (a b)"))
                        if sh == 0:
                            # sub-half 1's V32T chunks ride C(sh0)'s idle DVE
                            v32t_chunk(1, gc)
                        for gq in range(2):
                            av = vp_ps.tile([128, 4, O], F32, tag="vp",
                                            name="av")
                            for gg in range(4):
                                gi = 4 * gq + gg
                                for j in range(4):
                                    nc.tensor.matmul(
                                        av[32 * j:32 * (j + 1), gg, :],
                                        lhsT=te8[32 * j:32 * (j + 1), gi, :],
                                        rhs=V32T[32 * j:32 * (j + 1), :,
                                                 8 * gc + gi],
                                        start=True, stop=True,
                                        tile_position=(32 * j, 32 * j),
                                        skip_group_check=True)
                            nc.scalar.copy(
                                out=OUTo[:, 4 * gq:4 * gq + 4, :], in_=av)
                        nc.gpsimd.dma_start(out=outD[h, sh, gc], in_=OUTo)
    return nc


def _prep_inputs(q, k, query_weight, key_weight, value_weight):
    bf = ml_dtypes.bfloat16
    # combo[h, slot, p, kind, c, x]; kind 0=qT, 1=kT, 2=qw/16
    def pack_qk(x, core):
        # x[32, 4096, 256] -> core slice -> [h, slot, p, c, b]
        xc = x[:, core * BS_CORE:(core + 1) * BS_CORE, :]
        r = xc.reshape(NQ, 2, B_H, 2, 128)        # [s, h, b, c, p]
        return r.transpose(1, 0, 4, 3, 2)          # [h, s, p, c, b]

    qwp = (np.asarray(query_weight) / 16.0).reshape(NQ, 2, 128, A)
    qwp = qwp.transpose(0, 2, 1, 3)                # [s, p, c, a]

    kv = np.stack([np.asarray(key_weight), np.asarray(value_weight)], axis=2)
    # kv[s, d, w, a] -> [g4, p, s4, c, w, a]
    kvp = kv.reshape(8, 4, 2, 128, 2, A).transpose(0, 3, 1, 2, 4, 5)
    kvp = np.ascontiguousarray(kvp).astype(bf)

    in_maps = []
    for i in range(N_CORES):
        qp = pack_qk(np.asarray(q), i)
        kp = pack_qk(np.asarray(k), i)
        combo = np.empty((2, NQ, 128, 3, 2, 256), dtype=bf)
        combo[:, :, :, 0] = qp
        combo[:, :, :, 1] = kp
        combo[:, :, :, 2] = qwp[None]
        in_maps.append({"combo": combo, "kvw": kvp})
    return in_maps


def _unpack_out(outs):
    # per-core out [2 h, 2 sh, 4 gc, 128(32j+n), 8 gg, 256] ->
    # [nq, 512, 256] with b = h*256 + sh*128 + 32j + 8*gc + gg
    full = []
    for o in outs:
        od = np.asarray(o).reshape(2, 2, 4, 4, 32, 8, O)
        full.append(od.transpose(4, 0, 1, 3, 2, 5, 6).reshape(NQ, BS_CORE, O))
    return np.concatenate(full, axis=1).astype(np.float32)


_NC_CACHE = {}


def _get_nc():
    if "nc" not in _NC_CACHE:
        nc = build_kernel()
        nc.finalize()
        _NC_CACHE["nc"] = nc
    return _NC_CACHE["nc"]


def kernel(q, k, query_weight, key_weight, value_weight, _trace=False):
    nc = _get_nc()
    in_maps = _prep_inputs(q, k, query_weight, key_weight, value_weight)
    res = run_bass_kernel_spmd(nc, in_maps, core_ids=list(range(N_CORES)),
                               trace=_trace)
    full = _unpack_out([res.results[i]["out"] for i in range(N_CORES)])
    if _trace:
        return full, res
    return full
